# revision 9
# baseline (speedup 1.0000x reference)
"""Trainium2 Bass kernel for nn_EnhanceDiversityFeatureExtracition.

Computes  loss = mean((x-y)^2) + ALPHA * diversity_reg(conv_w)
where diversity_reg builds a 64x64 Gram matrix of the F=64 slices
conv_w[:, :, i, :] (each flattened to a 786432-vector), normalizes it to
cosine similarities, and sums the entries with tau < sim <= 1 off the
diagonal.

Distribution (8 NeuronCores, SPMD):
  - x_batch / y_batch sharded on batch dim: 256 rows per core.
  - conv_w viewed as A = conv_w.reshape(262144, 192); gram[i,j] =
    sum_m sum_k A[m,3i+k]*A[m,3j+k], so A is sharded along the 262144-row
    reduction axis: 32768 rows per core.
  - Each core returns a partial 64x64 gram and a per-partition partial
    sum of (x-y)^2 in one merged output tensor; the host sums the
    partials and applies the tiny 64x64 masked-similarity epilogue.

On-core dataflow (DMA-roofline bound: 32 MiB/core of HBM reads).
Transport findings (HW-measured on this part):
  - SWDGE (gpsimd) with 30KB per-partition lines sustains ~418 GB/s
    busy vs ~387 for HWDGE 12KB lines, but its first-descriptor
    emission idles the engines for ~5-6us at the start.
  - So: a short HWDGE (sync) prefix of three 8-tile mini-blocks covers
    the SWDGE emission ramp, then big SWDGE slices carry the bulk.
    x/y ride the same SWDGE queue as one 4MB transfer each (32KB
    lines), slotted between early conv slices.
  - Slice widths are multiples of the 192-col sub-tile so every matmul
    reads from a single pool tile; each tile carries 64 junk columns so
    the last sub-tile's 256-wide fp32r moving operand stays in bounds
    (junk only feeds PSUM columns 192:255, which are never read).
  - Slices shrink toward the end (40,...,2,1,1 tiles) so the PE drains
    ~0.3us after the last byte lands.
  - Per 128-row sub-tile: two fp32r full-rate matmuls accumulate
    C = A^T A into cps1/cps2 across the whole shard.
  - MSE: one DVE subtract (bf16 out) + one ACT Square accumulation
    into the output tile, mid-run.
  - Epilogue: cast C to bf16, six bf16 selection matmuls extract
    gram[i,j] = sum_k C[3i+k,3j+k], one merged DMA returns [128, 65]:
    cols 0:64 = gram rows (partitions 0:64), col 64 = MSE partials.
"""

import numpy as np
import ml_dtypes

import concourse.bass as bass
import concourse.mybir as mybir
from concourse import bacc, tile
from concourse.bass_utils import run_bass_kernel_spmd

N_CORES = 8
B, D = 2048, 4096
M, G = 262144, 192
F, KW = 64, 3
ROWS = B // N_CORES          # 256
MC = M // N_CORES            # 32768
NT = MC // 128               # 256 sub-tiles of [128, 192]
XCOLS = (ROWS * D) // 128    # 8192 x/y cols per partition
PAD = 64                     # junk cols for the fp32r 256-wide moving op

# HWDGE prefix mini-blocks (tiles each), then SWDGE slices (tiles each)
HPRE = [8, 8, 8]
SLICES = [36, 36, 36, 36, 36, 24, 12, 8, 4, 2, 1, 1]
assert sum(HPRE) + sum(SLICES) == NT

ALPHA = 0.0005
TAU = 0.2

_prog = None


def _build() -> bass.Bass:
    nc = bacc.Bacc(None, target_bir_lowering=False)
    f32 = mybir.dt.float32
    f32r = mybir.dt.float32r
    bf16 = mybir.dt.bfloat16

    xs = nc.dram_tensor("xs", [ROWS, D], f32, kind="ExternalInput")
    ys = nc.dram_tensor("ys", [ROWS, D], f32, kind="ExternalInput")
    aw = nc.dram_tensor("aw", [MC, G], f32r, kind="ExternalInput")
    out = nc.dram_tensor("out", [128, F + 1], f32, kind="ExternalOutput")

    # Selection matrix (bf16, exact 0/1):
    # gram[i,j] = sum_k C[3i+k, 3j+k] via S[3j+k, 64k+j] = 1
    S = np.zeros((G, G), np.float32)
    for k in range(KW):
        for j in range(F):
            S[KW * j + k, F * k + j] = 1.0
    s_dram = nc.inline_tensor(S.astype(ml_dtypes.bfloat16), name="sel_const")

    RW = 256  # moving operand width for the fp32r full-rate mode

    with tile.TileContext(nc) as tc:
        with (
            tc.tile_pool(name="apool", bufs=3) as apool,
            tc.tile_pool(name="hpool", bufs=3) as hpool,
            tc.tile_pool(name="xpool", bufs=1) as xpool,
            tc.tile_pool(name="ypool", bufs=1) as ypool,
            tc.tile_pool(name="dpool", bufs=1) as dpool,
            tc.tile_pool(name="opool", bufs=1) as opool,
            tc.tile_pool(name="spool", bufs=1) as spool,
            tc.tile_pool(name="psum", bufs=1, space=bass.MemorySpace.PSUM) as psum,
        ):
            cps1 = psum.tile([128, RW], f32, tag="cps1")
            cps2 = psum.tile([F, RW], f32, tag="cps2")
            otile = opool.tile([128, F + 1], f32, tag="otile")

            ssb1 = spool.tile([128, G], bf16, tag="ssb1")
            nc.sync.dma_start(ssb1[:], s_dram[0:128, :])
            ssb2 = spool.tile([F, G], bf16, tag="ssb2")
            nc.sync.dma_start(ssb2[:], s_dram[128:G, :])

            awv = aw[:].rearrange("(p t) g -> p (t g)", p=128)
            xv = xs[:].rearrange("(p t) d -> p (t d)", p=128)
            yv = ys[:].rearrange("(p t) d -> p (t d)", p=128)

            xt = xpool.tile([128, XCOLS], f32, tag="xt")
            yt = ypool.tile([128, XCOLS], f32, tag="yt")

            # issue every DMA up front: HWDGE prefix on sync, bulk on the
            # SWDGE queue with x/y slotted between early conv slices
            blocks = []  # (tile_handle, n_tiles) in consumption order
            c0 = 0
            for nt_ in HPRE:
                ht = hpool.tile([128, 8 * G + PAD], f32r, tag="hpre")
                nc.sync.dma_start(ht[:, :nt_ * G], awv[:, c0:c0 + nt_ * G])
                blocks.append((ht, nt_))
                c0 += nt_ * G
            for i, nt_ in enumerate(SLICES):
                at = apool.tile([128, 36 * G + PAD], f32r, tag="at")
                nc.gpsimd.dma_start(at[:, :nt_ * G], awv[:, c0:c0 + nt_ * G])
                blocks.append((at, nt_))
                c0 += nt_ * G
                if i == 0:
                    nc.gpsimd.dma_start(xt[:], xv[:])
                if i == 1:
                    nc.gpsimd.dma_start(yt[:], yv[:])

            # MSE: d = x - y (bf16 out), one Square accumulation
            dtile = dpool.tile([128, XCOLS], bf16, tag="dt")
            nc.vector.tensor_sub(dtile[:], xt[:], yt[:])
            nc.scalar.activation(
                dtile[:], dtile[:],
                mybir.ActivationFunctionType.Square,
                accum_out=otile[:, F:F + 1],
            )

            # C = A^T A: two fp32r full-rate matmuls per sub-tile
            ti = 0
            for bt, nt_ in blocks:
                for t in range(nt_):
                    rhs = bt[:, t * G:t * G + RW]
                    w1 = bt[:, t * G:t * G + 128]
                    w2 = bt[:, t * G + 128:t * G + G]
                    nc.tensor.matmul(
                        cps1[:], w1, rhs,
                        start=(ti == 0), stop=(ti == NT - 1),
                    )
                    nc.tensor.matmul(
                        cps2[:], w2, rhs,
                        start=(ti == 0), stop=(ti == NT - 1),
                    )
                    ti += 1

            # epilogue: cast C to bf16, six selection matmuls, merged out
            csb1 = opool.tile([128, G], bf16, tag="csb1")
            nc.vector.tensor_copy(csb1[:], cps1[:, :G])
            csb2 = opool.tile([F, G], bf16, tag="csb2")
            nc.vector.tensor_copy(csb2[:], cps2[:, :G])

            gps = psum.tile([F, F], f32, tag="gps")
            gi = 0
            for k in range(KW):
                for ssb, csb in ((ssb1, csb1), (ssb2, csb2)):
                    nc.tensor.matmul(
                        gps[:],
                        ssb[:, F * k:F * (k + 1)],
                        csb[:, k::KW],
                        start=(gi == 0), stop=(gi == 2 * KW - 1),
                    )
                    gi += 1

            nc.vector.tensor_copy(otile[0:F, 0:F], gps[:])
            nc.sync.dma_start(out[:], otile[:])

    nc.finalize()
    return nc


def _get_prog() -> bass.Bass:
    global _prog
    if _prog is None:
        _prog = _build()
    return _prog


def _epilogue(gram: np.ndarray, sse: float) -> np.ndarray:
    norms = np.sqrt(np.diag(gram))
    sim = gram / np.outer(norms, norms)
    mask = (sim > TAU) & (sim <= 1.0) & (~np.eye(F, dtype=bool))
    reg = sim[mask].sum()
    loss = sse / float(B * D) + ALPHA * reg
    return np.asarray(np.float32(loss))


def kernel(x_batch: np.ndarray, y_batch: np.ndarray, conv_w: np.ndarray) -> np.ndarray:
    nc = _get_prog()
    A = np.ascontiguousarray(conv_w.reshape(M, G))
    in_maps = []
    for c in range(N_CORES):
        in_maps.append({
            "xs": np.ascontiguousarray(x_batch[c * ROWS:(c + 1) * ROWS]),
            "ys": np.ascontiguousarray(y_batch[c * ROWS:(c + 1) * ROWS]),
            "aw": np.ascontiguousarray(A[c * MC:(c + 1) * MC]),
        })
    res = run_bass_kernel_spmd(nc, in_maps, core_ids=list(range(N_CORES))).results
    gram = np.zeros((F, F), np.float64)
    sse = 0.0
    for r in res:
        o = r["out"]
        gram += o[:F, :F].astype(np.float64)
        sse += float(o[:, F].sum(dtype=np.float64))
    return _epilogue(gram, sse)


# revision 10
# speedup vs baseline: 1.0413x; 1.0413x over previous
"""Trainium2 Bass kernel for nn_EnhanceDiversityFeatureExtracition.

Computes  loss = mean((x-y)^2) + ALPHA * diversity_reg(conv_w)
where diversity_reg builds a 64x64 Gram matrix of the F=64 slices
conv_w[:, :, i, :] (each flattened to a 786432-vector), normalizes it to
cosine similarities, and sums the entries with tau < sim <= 1 off the
diagonal.

Distribution (8 NeuronCores, SPMD):
  - x_batch / y_batch sharded on batch dim: 256 rows per core.
  - conv_w viewed as A = conv_w.reshape(262144, 192); gram[i,j] =
    sum_m sum_k A[m,3i+k]*A[m,3j+k], so A is sharded along the 262144-row
    reduction axis: 32768 rows per core.
  - Each core returns a partial 64x64 gram and a per-partition partial
    sum of (x-y)^2 in one merged output tensor; the host sums the
    partials and applies the tiny 64x64 masked-similarity epilogue.

On-core dataflow (DMA-roofline bound: 32 MiB/core of HBM reads).
Transport findings (HW-measured on this part):
  - SWDGE (gpsimd) with 30KB per-partition lines sustains ~418 GB/s
    busy vs ~387 for HWDGE 12KB lines, but its first-descriptor
    emission idles the engines for ~5-6us at the start.
  - HWDGE and SWDGE queues must NOT be mixed mid-stream: the SDMA
    engines switch queues only at coarse packet boundaries, so a
    deeply-queued SWDGE stream starves HWDGE transfers for tens of us
    (measured: a 768KB HWDGE block took ~25us under SWDGE load).
    Everything bulk rides the single SWDGE queue; only the tiny
    selection-matrix load and the final output use the sync ring.
    x/y ride the same SWDGE queue as one 4MB transfer each (32KB
    lines), slotted between early conv slices.
  - Slice widths are multiples of the 192-col sub-tile so every matmul
    reads from a single pool tile; each tile carries 64 junk columns so
    the last sub-tile's 256-wide fp32r moving operand stays in bounds
    (junk only feeds PSUM columns 192:255, which are never read).
  - Slices shrink toward the end (40,...,2,1,1 tiles) so the PE drains
    ~0.3us after the last byte lands.
  - Per 128-row sub-tile: two fp32r full-rate matmuls accumulate
    C = A^T A into cps1/cps2 across the whole shard.
  - MSE: one DVE subtract (bf16 out) + one ACT Square accumulation
    into the output tile, mid-run.
  - Epilogue: cast C to bf16, six bf16 selection matmuls extract
    gram[i,j] = sum_k C[3i+k,3j+k], one merged DMA returns [128, 65]:
    cols 0:64 = gram rows (partitions 0:64), col 64 = MSE partials.
"""

import numpy as np
import ml_dtypes

import concourse.bass as bass
import concourse.mybir as mybir
from concourse import bacc, tile
from concourse.bass_utils import run_bass_kernel_spmd

N_CORES = 8
B, D = 2048, 4096
M, G = 262144, 192
F, KW = 64, 3
ROWS = B // N_CORES          # 256
MC = M // N_CORES            # 32768
NT = MC // 128               # 256 sub-tiles of [128, 192]
XCOLS = (ROWS * D) // 128    # 8192 x/y cols per partition
PAD = 64                     # junk cols for the fp32r 256-wide moving op

# SWDGE slices (tiles each): 24KB lines (6x4KB) for the bulk, shrinking
# tail so the PE drains right after the last byte
SLICES = [32, 32, 32, 32, 32, 32, 32, 12, 8, 6, 3, 2, 1]
assert sum(SLICES) == NT

ALPHA = 0.0005
TAU = 0.2

_prog = None


def _build() -> bass.Bass:
    nc = bacc.Bacc(None, target_bir_lowering=False)
    f32 = mybir.dt.float32
    f32r = mybir.dt.float32r
    bf16 = mybir.dt.bfloat16

    xs = nc.dram_tensor("xs", [ROWS, D], f32, kind="ExternalInput")
    ys = nc.dram_tensor("ys", [ROWS, D], f32, kind="ExternalInput")
    aw = nc.dram_tensor("aw", [MC, G], f32r, kind="ExternalInput")
    out = nc.dram_tensor("out", [128, F + 1], f32, kind="ExternalOutput")

    # Selection matrix (bf16, exact 0/1):
    # gram[i,j] = sum_k C[3i+k, 3j+k] via S[3j+k, 64k+j] = 1
    S = np.zeros((G, G), np.float32)
    for k in range(KW):
        for j in range(F):
            S[KW * j + k, F * k + j] = 1.0
    s_dram = nc.inline_tensor(S.astype(ml_dtypes.bfloat16), name="sel_const")

    RW = 256  # moving operand width for the fp32r full-rate mode

    with tile.TileContext(nc) as tc:
        with (
            tc.tile_pool(name="apool", bufs=4) as apool,
            tc.tile_pool(name="xpool", bufs=1) as xpool,
            tc.tile_pool(name="ypool", bufs=1) as ypool,
            tc.tile_pool(name="dpool", bufs=1) as dpool,
            tc.tile_pool(name="opool", bufs=1) as opool,
            tc.tile_pool(name="spool", bufs=1) as spool,
            tc.tile_pool(name="psum", bufs=1, space=bass.MemorySpace.PSUM) as psum,
        ):
            cps1 = psum.tile([128, RW], f32, tag="cps1")
            cps2 = psum.tile([F, RW], f32, tag="cps2")
            otile = opool.tile([128, F + 1], f32, tag="otile")

            ssb1 = spool.tile([128, G], bf16, tag="ssb1")
            nc.sync.dma_start(ssb1[:], s_dram[0:128, :])
            ssb2 = spool.tile([F, G], bf16, tag="ssb2")
            nc.sync.dma_start(ssb2[:], s_dram[128:G, :])

            awv = aw[:].rearrange("(p t) g -> p (t g)", p=128)
            xv = xs[:].rearrange("(p t) d -> p (t d)", p=128)
            yv = ys[:].rearrange("(p t) d -> p (t d)", p=128)

            xt = xpool.tile([128, XCOLS], f32, tag="xt")
            yt = ypool.tile([128, XCOLS], f32, tag="yt")

            # issue every DMA up front: HWDGE prefix on sync, bulk on the
            # SWDGE queue with x/y slotted between early conv slices
            blocks = []  # (tile_handle, n_tiles) in consumption order
            c0 = 0
            for i, nt_ in enumerate(SLICES):
                at = apool.tile([128, 32 * G + PAD], f32r, tag="at")
                nc.gpsimd.dma_start(at[:, :nt_ * G], awv[:, c0:c0 + nt_ * G])
                blocks.append((at, nt_))
                c0 += nt_ * G
                if i == 0:
                    nc.gpsimd.dma_start(xt[:], xv[:])
                if i == 1:
                    nc.gpsimd.dma_start(yt[:], yv[:])

            # MSE: d = x - y (bf16 out), one Square accumulation
            dtile = dpool.tile([128, XCOLS], bf16, tag="dt")
            nc.vector.tensor_sub(dtile[:], xt[:], yt[:])
            nc.scalar.activation(
                dtile[:], dtile[:],
                mybir.ActivationFunctionType.Square,
                accum_out=otile[:, F:F + 1],
            )

            # C = A^T A: two fp32r full-rate matmuls per sub-tile
            ti = 0
            for bt, nt_ in blocks:
                for t in range(nt_):
                    rhs = bt[:, t * G:t * G + RW]
                    w1 = bt[:, t * G:t * G + 128]
                    w2 = bt[:, t * G + 128:t * G + G]
                    nc.tensor.matmul(
                        cps1[:], w1, rhs,
                        start=(ti == 0), stop=(ti == NT - 1),
                    )
                    nc.tensor.matmul(
                        cps2[:], w2, rhs,
                        start=(ti == 0), stop=(ti == NT - 1),
                    )
                    ti += 1

            # epilogue: cast C to bf16, six selection matmuls, merged out
            csb1 = opool.tile([128, G], bf16, tag="csb1")
            nc.vector.tensor_copy(csb1[:], cps1[:, :G])
            csb2 = opool.tile([F, G], bf16, tag="csb2")
            nc.vector.tensor_copy(csb2[:], cps2[:, :G])

            gps = psum.tile([F, F], f32, tag="gps")
            gi = 0
            for k in range(KW):
                for ssb, csb in ((ssb1, csb1), (ssb2, csb2)):
                    nc.tensor.matmul(
                        gps[:],
                        ssb[:, F * k:F * (k + 1)],
                        csb[:, k::KW],
                        start=(gi == 0), stop=(gi == 2 * KW - 1),
                    )
                    gi += 1

            nc.vector.tensor_copy(otile[0:F, 0:F], gps[:])
            nc.sync.dma_start(out[:], otile[:])

    nc.finalize()
    return nc


def _get_prog() -> bass.Bass:
    global _prog
    if _prog is None:
        _prog = _build()
    return _prog


def _epilogue(gram: np.ndarray, sse: float) -> np.ndarray:
    norms = np.sqrt(np.diag(gram))
    sim = gram / np.outer(norms, norms)
    mask = (sim > TAU) & (sim <= 1.0) & (~np.eye(F, dtype=bool))
    reg = sim[mask].sum()
    loss = sse / float(B * D) + ALPHA * reg
    return np.asarray(np.float32(loss))


def kernel(x_batch: np.ndarray, y_batch: np.ndarray, conv_w: np.ndarray) -> np.ndarray:
    nc = _get_prog()
    A = np.ascontiguousarray(conv_w.reshape(M, G))
    in_maps = []
    for c in range(N_CORES):
        in_maps.append({
            "xs": np.ascontiguousarray(x_batch[c * ROWS:(c + 1) * ROWS]),
            "ys": np.ascontiguousarray(y_batch[c * ROWS:(c + 1) * ROWS]),
            "aw": np.ascontiguousarray(A[c * MC:(c + 1) * MC]),
        })
    res = run_bass_kernel_spmd(nc, in_maps, core_ids=list(range(N_CORES))).results
    gram = np.zeros((F, F), np.float64)
    sse = 0.0
    for r in res:
        o = r["out"]
        gram += o[:F, :F].astype(np.float64)
        sse += float(o[:, F].sum(dtype=np.float64))
    return _epilogue(gram, sse)


# revision 11
# speedup vs baseline: 1.0418x; 1.0005x over previous
"""Trainium2 Bass kernel for nn_EnhanceDiversityFeatureExtracition.

Computes  loss = mean((x-y)^2) + ALPHA * diversity_reg(conv_w)
where diversity_reg builds a 64x64 Gram matrix of the F=64 slices
conv_w[:, :, i, :] (each flattened to a 786432-vector), normalizes it to
cosine similarities, and sums the entries with tau < sim <= 1 off the
diagonal.

Distribution (8 NeuronCores, SPMD):
  - x_batch / y_batch sharded on batch dim: 256 rows per core.
  - conv_w viewed as A = conv_w.reshape(262144, 192); gram[i,j] =
    sum_m sum_k A[m,3i+k]*A[m,3j+k], so A is sharded along the 262144-row
    reduction axis: 32768 rows per core.
  - Each core returns a partial 64x64 gram and per-partition partial
    sums of (x-y)^2 in one merged output tensor; the host sums the
    partials and applies the tiny 64x64 masked-similarity epilogue.

On-core dataflow (DMA-roofline bound: 32 MiB/core of HBM reads).
Transport notes (HW-measured on this part):
  - One HWDGE ring (sync), one logical queue, for EVERY transfer.
    Splitting across rings/queues or using SWDGE for the bulk measured
    strictly worse in-context (engines switch queues at coarse packet
    boundaries; a deeply queued SWDGE stream starves everything else,
    and SWDGE itself drops to ~20 GB/s/engine under PE load vs 26 in
    isolation; HWDGE sustains ~24.5 in-context).
  - Conv streams as [128 x 1536] blocks (6KB lines); x/y as
    [128 x 2048] chunks (8KB lines), one per conv block over the first
    16 blocks so the MSE traffic is spread and the tensor engine never
    idles past the HAM 3.4us clock-gate window.
  - Each conv tile carries 64 junk columns so the last sub-tile's
    256-wide fp32r moving operand stays in bounds; junk only feeds
    PSUM columns 192:255, which are never read.  No memset.
  - The final conv blocks shrink (8,...,4,2,2 tiles) so the PE drains
    right after the last byte lands.
  - Per 128-row sub-tile: two fp32r full-rate matmuls accumulate
    C = A^T A into cps1/cps2 across the whole shard.
  - Epilogue: cast C to bf16, six bf16 selection matmuls extract
    gram[i,j] = sum_k C[3i+k,3j+k] (exact 0/1 selection matrix), one
    merged DMA returns [128, 72]: cols 0:64 = gram rows (partitions
    0:64), cols 64:72 = MSE partial sums.
"""

import numpy as np
import ml_dtypes

import concourse.bass as bass
import concourse.mybir as mybir
from concourse import bacc, tile
from concourse.bass_utils import run_bass_kernel_spmd

N_CORES = 8
B, D = 2048, 4096
M, G = 262144, 192
F, KW = 64, 3
ROWS = B // N_CORES          # 256
MC = M // N_CORES            # 32768
NT = MC // 128               # 256 sub-tiles of [128, 192]
PAD = 64                     # junk cols for the fp32r 256-wide moving op
TPBS = [8] * 31 + [4, 2, 2]  # tiles per conv block, shrinking tail
assert sum(TPBS) == NT
NCH = 8                      # MSE chunks per operand
CHW = (ROWS * D) // (128 * NCH)  # 1024 cols per partition per chunk

ALPHA = 0.0005
TAU = 0.2

_prog = None


def _build() -> bass.Bass:
    nc = bacc.Bacc(None, target_bir_lowering=False)
    f32 = mybir.dt.float32
    f32r = mybir.dt.float32r
    bf16 = mybir.dt.bfloat16

    xs = nc.dram_tensor("xs", [ROWS, D], f32, kind="ExternalInput")
    ys = nc.dram_tensor("ys", [ROWS, D], f32, kind="ExternalInput")
    aw = nc.dram_tensor("aw", [MC, G], f32r, kind="ExternalInput")
    out = nc.dram_tensor("out", [128, F + NCH], f32, kind="ExternalOutput")

    # Selection matrix (bf16, exact 0/1):
    # gram[i,j] = sum_k C[3i+k, 3j+k] via S[3j+k, 64k+j] = 1
    S = np.zeros((G, G), np.float32)
    for k in range(KW):
        for j in range(F):
            S[KW * j + k, F * k + j] = 1.0
    s_dram = nc.inline_tensor(S.astype(ml_dtypes.bfloat16), name="sel_const")

    RW = 256  # moving operand width for the fp32r full-rate mode

    with tile.TileContext(nc) as tc:
        with (
            tc.tile_pool(name="apool", bufs=12) as apool,
            tc.tile_pool(name="xpool", bufs=2) as xpool,
            tc.tile_pool(name="ypool", bufs=2) as ypool,
            tc.tile_pool(name="dpool", bufs=2) as dpool,
            tc.tile_pool(name="opool", bufs=1) as opool,
            tc.tile_pool(name="spool", bufs=1) as spool,
            tc.tile_pool(name="psum", bufs=1, space=bass.MemorySpace.PSUM) as psum,
        ):
            cps1 = psum.tile([128, RW], f32, tag="cps1")
            cps2 = psum.tile([F, RW], f32, tag="cps2")
            otile = opool.tile([128, F + NCH], f32, tag="otile")

            awv = aw[:].rearrange("(p t) g -> p (t g)", p=128)
            xv = xs[:].rearrange("(p t) d -> p (t d)", p=128)
            yv = ys[:].rearrange("(p t) d -> p (t d)", p=128)

            ssb1 = spool.tile([128, G], bf16, tag="ssb1")
            ssb2 = spool.tile([F, G], bf16, tag="ssb2")

            ti = 0
            c0 = 0
            for b, tpb in enumerate(TPBS):
                at = apool.tile([128, 8 * G + PAD], f32r, tag="at")
                nc.sync.dma_start(at[:, :tpb * G], awv[:, c0:c0 + tpb * G])
                c0 += tpb * G

                # one x or y chunk per block over the first 16 blocks,
                # sel matrix right after conv block 0
                if b == 0:
                    nc.sync.dma_start(ssb1[:], s_dram[0:128, :])
                    nc.sync.dma_start(ssb2[:], s_dram[128:G, :])
                if b < 2 * NCH:
                    ch = b // 2
                    if b % 2 == 0:
                        xt = xpool.tile([128, CHW], f32, tag="xt")
                        nc.sync.dma_start(xt[:], xv[:, ch * CHW:(ch + 1) * CHW])
                    else:
                        yt = ypool.tile([128, CHW], f32, tag="yt")
                        nc.sync.dma_start(yt[:], yv[:, ch * CHW:(ch + 1) * CHW])

                for t in range(tpb):
                    rhs = at[:, t * G:t * G + RW]
                    w1 = at[:, t * G:t * G + 128]
                    w2 = at[:, t * G + 128:t * G + G]
                    nc.tensor.matmul(
                        cps1[:], w1, rhs,
                        start=(ti == 0), stop=(ti == NT - 1),
                    )
                    nc.tensor.matmul(
                        cps2[:], w2, rhs,
                        start=(ti == 0), stop=(ti == NT - 1),
                    )
                    ti += 1

                if b < 2 * NCH and b % 2 == 1:
                    ch = b // 2
                    dtile = dpool.tile([128, CHW], f32, tag="dt")
                    nc.vector.tensor_sub(dtile[:], xt[:], yt[:])
                    nc.scalar.activation(
                        dtile[:], dtile[:],
                        mybir.ActivationFunctionType.Square,
                        accum_out=otile[:, F + ch:F + ch + 1],
                    )

            # epilogue: cast C to bf16, six selection matmuls, merged out
            csb1 = opool.tile([128, G], bf16, tag="csb1")
            nc.vector.tensor_copy(csb1[:], cps1[:, :G])
            csb2 = opool.tile([F, G], bf16, tag="csb2")
            nc.vector.tensor_copy(csb2[:], cps2[:, :G])

            gps = psum.tile([F, F], f32, tag="gps")
            gi = 0
            for k in range(KW):
                for ssb, csb in ((ssb1, csb1), (ssb2, csb2)):
                    nc.tensor.matmul(
                        gps[:],
                        ssb[:, F * k:F * (k + 1)],
                        csb[:, k::KW],
                        start=(gi == 0), stop=(gi == 2 * KW - 1),
                    )
                    gi += 1

            nc.vector.tensor_copy(otile[0:F, 0:F], gps[:])
            nc.sync.dma_start(out[:], otile[:])

    nc.finalize()
    return nc


def _get_prog() -> bass.Bass:
    global _prog
    if _prog is None:
        _prog = _build()
    return _prog


def _epilogue(gram: np.ndarray, sse: float) -> np.ndarray:
    norms = np.sqrt(np.diag(gram))
    sim = gram / np.outer(norms, norms)
    mask = (sim > TAU) & (sim <= 1.0) & (~np.eye(F, dtype=bool))
    reg = sim[mask].sum()
    loss = sse / float(B * D) + ALPHA * reg
    return np.asarray(np.float32(loss))


def kernel(x_batch: np.ndarray, y_batch: np.ndarray, conv_w: np.ndarray) -> np.ndarray:
    nc = _get_prog()
    A = np.ascontiguousarray(conv_w.reshape(M, G))
    in_maps = []
    for c in range(N_CORES):
        in_maps.append({
            "xs": np.ascontiguousarray(x_batch[c * ROWS:(c + 1) * ROWS]),
            "ys": np.ascontiguousarray(y_batch[c * ROWS:(c + 1) * ROWS]),
            "aw": np.ascontiguousarray(A[c * MC:(c + 1) * MC]),
        })
    res = run_bass_kernel_spmd(nc, in_maps, core_ids=list(range(N_CORES))).results
    gram = np.zeros((F, F), np.float64)
    sse = 0.0
    for r in res:
        o = r["out"]
        gram += o[:F, :F].astype(np.float64)
        sse += float(o[:, F:F + NCH].sum(dtype=np.float64))
    return _epilogue(gram, sse)


# revision 12
# speedup vs baseline: 1.0701x; 1.0271x over previous
"""Trainium2 Bass kernel for nn_EnhanceDiversityFeatureExtracition.

Computes  loss = mean((x-y)^2) + ALPHA * diversity_reg(conv_w)
where diversity_reg builds a 64x64 Gram matrix of the F=64 slices
conv_w[:, :, i, :] (each flattened to a 786432-vector), normalizes it to
cosine similarities, and sums the entries with tau < sim <= 1 off the
diagonal.

Distribution (8 NeuronCores, SPMD):
  - x_batch / y_batch sharded on batch dim: 256 rows per core.
  - conv_w viewed as A = conv_w.reshape(262144, 192); gram[i,j] =
    sum_m sum_k A[m,3i+k]*A[m,3j+k], so A is sharded along the 262144-row
    reduction axis: 32768 rows per core.
  - Each core computes the full partial C = A_c^T A_c (192x192) plus
    per-partition partial sums of (x-y)^2 and returns both in one
    merged output tensor.  The host sums the C partials, gathers
    gram[i,j] = sum_k C[3i+k, 3j+k] (a 36K-element reindex), and
    applies the tiny 64x64 masked-similarity epilogue it already owns.
    Shipping C instead of gram deletes the on-device selection-matmul
    epilogue from the critical-path tail (~2us).

On-core dataflow (DMA-roofline bound: 32 MiB/core of HBM reads).
Transport notes (HW-measured on this part):
  - One HWDGE ring (sync), one logical queue, for EVERY transfer.
    Splitting across rings/queues or using SWDGE for the bulk measured
    strictly worse in-context (engines switch queues at coarse packet
    boundaries; a deeply queued SWDGE stream starves everything else,
    and SWDGE itself drops to ~20 GB/s/engine under PE load vs 26 in
    isolation; HWDGE sustains ~24.5 in-context).
  - Conv streams as [128 x 3072] blocks (12KB lines = 3x4KB packets,
    measured ~3% faster per engine than 6KB lines); x/y as
    [128 x 2048] chunks (8KB lines, the fastest measured line size),
    one per conv block over the first 8 blocks so the MSE traffic is
    spread and the tensor engine never idles past the HAM 3.4us
    clock-gate window.
  - Each conv tile carries 64 junk columns so the last sub-tile's
    256-wide fp32r moving operand stays in bounds; junk only feeds
    PSUM columns 192:255, which are never read.  No memset.
  - The final conv blocks shrink (16,...,8,4,2,2 tiles) so the PE
    drains right after the last byte lands.
  - Per 128-row sub-tile: two fp32r full-rate matmuls accumulate
    C = A^T A into cps1/cps2 across the whole shard.
  - Tail: two DVE copies move C out of PSUM into the merged output
    tile, then a single DMA returns [128, 388]: cols 0:192 = C rows
    0:128, cols 192:384 = C rows 128:192 (partitions 0:64), cols
    384:388 = MSE partial sums.
"""

import numpy as np

import concourse.bass as bass
import concourse.mybir as mybir
from concourse import bacc, tile
from concourse.bass_utils import run_bass_kernel_spmd

N_CORES = 8
B, D = 2048, 4096
M, G = 262144, 192
F, KW = 64, 3
ROWS = B // N_CORES          # 256
MC = M // N_CORES            # 32768
NT = MC // 128               # 256 sub-tiles of [128, 192]
PAD = 64                     # junk cols for the fp32r 256-wide moving op
TPBS = [16] * 14 + [8, 8, 8, 4, 2, 2]  # tiles per conv block
assert sum(TPBS) == NT
NCH = 4                      # MSE chunks per operand
CHW = (ROWS * D) // (128 * NCH)  # 2048 cols per partition per chunk
OC = 2 * G + NCH             # merged output cols: C(0:128), C(128:192), sse

ALPHA = 0.0005
TAU = 0.2

_prog = None


def _build() -> bass.Bass:
    nc = bacc.Bacc(None, target_bir_lowering=False)
    f32 = mybir.dt.float32
    f32r = mybir.dt.float32r

    xs = nc.dram_tensor("xs", [ROWS, D], f32, kind="ExternalInput")
    ys = nc.dram_tensor("ys", [ROWS, D], f32, kind="ExternalInput")
    aw = nc.dram_tensor("aw", [MC, G], f32r, kind="ExternalInput")
    out = nc.dram_tensor("out", [128, OC], f32, kind="ExternalOutput")

    RW = 256  # moving operand width for the fp32r full-rate mode

    with tile.TileContext(nc) as tc:
        with (
            tc.tile_pool(name="apool", bufs=8) as apool,
            tc.tile_pool(name="xpool", bufs=2) as xpool,
            tc.tile_pool(name="ypool", bufs=2) as ypool,
            tc.tile_pool(name="dpool", bufs=2) as dpool,
            tc.tile_pool(name="opool", bufs=1) as opool,
            tc.tile_pool(name="psum", bufs=1, space=bass.MemorySpace.PSUM) as psum,
        ):
            cps1 = psum.tile([128, RW], f32, tag="cps1")
            cps2 = psum.tile([F, RW], f32, tag="cps2")
            otile = opool.tile([128, OC], f32, tag="otile")

            awv = aw[:].rearrange("(p t) g -> p (t g)", p=128)
            xv = xs[:].rearrange("(p t) d -> p (t d)", p=128)
            yv = ys[:].rearrange("(p t) d -> p (t d)", p=128)

            ti = 0
            c0 = 0
            for b, tpb in enumerate(TPBS):
                at = apool.tile([128, 16 * G + PAD], f32r, tag="at")
                nc.sync.dma_start(at[:, :tpb * G], awv[:, c0:c0 + tpb * G])
                c0 += tpb * G

                # one x or y chunk per block over the first 8 blocks
                if b < 2 * NCH:
                    ch = b // 2
                    if b % 2 == 0:
                        xt = xpool.tile([128, CHW], f32, tag="xt")
                        nc.sync.dma_start(xt[:], xv[:, ch * CHW:(ch + 1) * CHW])
                    else:
                        yt = ypool.tile([128, CHW], f32, tag="yt")
                        nc.sync.dma_start(yt[:], yv[:, ch * CHW:(ch + 1) * CHW])

                for t in range(tpb):
                    rhs = at[:, t * G:t * G + RW]
                    w1 = at[:, t * G:t * G + 128]
                    w2 = at[:, t * G + 128:t * G + G]
                    nc.tensor.matmul(
                        cps1[:], w1, rhs,
                        start=(ti == 0), stop=(ti == NT - 1),
                    )
                    nc.tensor.matmul(
                        cps2[:], w2, rhs,
                        start=(ti == 0), stop=(ti == NT - 1),
                    )
                    ti += 1

                if b < 2 * NCH and b % 2 == 1:
                    ch = b // 2
                    dtile = dpool.tile([128, CHW], f32, tag="dt")
                    nc.vector.tensor_sub(dtile[:], xt[:], yt[:])
                    nc.scalar.activation(
                        dtile[:], dtile[:],
                        mybir.ActivationFunctionType.Square,
                        accum_out=otile[:, 2 * G + ch:2 * G + ch + 1],
                    )

            # tail: move C out of PSUM and ship it raw; the host does the
            # k-diagonal gather
            nc.vector.tensor_copy(otile[:, 0:G], cps1[:, :G])
            nc.vector.tensor_copy(otile[0:F, G:2 * G], cps2[:, :G])
            nc.sync.dma_start(out[:], otile[:])

    nc.finalize()
    return nc


def _get_prog() -> bass.Bass:
    global _prog
    if _prog is None:
        _prog = _build()
    return _prog


def _epilogue(C: np.ndarray, sse: float) -> np.ndarray:
    # gram[i,j] = sum_k C[3i+k, 3j+k]
    Cr = C.reshape(F, KW, F, KW)
    gram = sum(Cr[:, k, :, k] for k in range(KW))
    norms = np.sqrt(np.diag(gram))
    sim = gram / np.outer(norms, norms)
    mask = (sim > TAU) & (sim <= 1.0) & (~np.eye(F, dtype=bool))
    reg = sim[mask].sum()
    loss = sse / float(B * D) + ALPHA * reg
    return np.asarray(np.float32(loss))


def kernel(x_batch: np.ndarray, y_batch: np.ndarray, conv_w: np.ndarray) -> np.ndarray:
    nc = _get_prog()
    A = np.ascontiguousarray(conv_w.reshape(M, G))
    in_maps = []
    for c in range(N_CORES):
        in_maps.append({
            "xs": np.ascontiguousarray(x_batch[c * ROWS:(c + 1) * ROWS]),
            "ys": np.ascontiguousarray(y_batch[c * ROWS:(c + 1) * ROWS]),
            "aw": np.ascontiguousarray(A[c * MC:(c + 1) * MC]),
        })
    res = run_bass_kernel_spmd(nc, in_maps, core_ids=list(range(N_CORES))).results
    C = np.zeros((G, G), np.float64)
    sse = 0.0
    for r in res:
        o = r["out"]
        C[0:128] += o[:, 0:G].astype(np.float64)
        C[128:G] += o[0:F, G:2 * G].astype(np.float64)
        sse += float(o[:, 2 * G:2 * G + NCH].sum(dtype=np.float64))
    return _epilogue(C, sse)


# revision 13
# speedup vs baseline: 1.2077x; 1.1286x over previous
"""Trainium2 Bass kernel for nn_EnhanceDiversityFeatureExtracition.

Computes  loss = mean((x-y)^2) + ALPHA * diversity_reg(conv_w)
where diversity_reg builds a 64x64 Gram matrix of the F=64 slices
conv_w[:, :, i, :] (each flattened to a 786432-vector), normalizes it to
cosine similarities, and sums the entries with tau < sim <= 1 off the
diagonal.

Distribution (8 NeuronCores, SPMD):
  - x_batch / y_batch sharded on batch dim: 256 rows per core.
  - conv_w viewed as A = conv_w.reshape(262144, 192); gram[i,j] =
    sum_m sum_k A[m,3i+k]*A[m,3j+k], so A is sharded along the 262144-row
    reduction axis: 32768 rows per core.
  - Each core computes the full partial C = A_c^T A_c (192x192) plus
    per-partition partial sums of (x-y)^2 and returns both in one
    merged output tensor.  The host sums the C partials, gathers
    gram[i,j] = sum_k C[3i+k, 3j+k] (a 36K-element reindex), and
    applies the tiny 64x64 masked-similarity epilogue it already owns.
    Shipping C instead of gram deletes the on-device selection-matmul
    epilogue from the critical-path tail (~2us).

On-core dataflow (DMA-roofline bound: 32 MiB/core of HBM reads).
Transport notes (HW-measured on this part):
  - One HWDGE ring (sync), one logical queue, for EVERY transfer.
    Splitting across rings/queues or using SWDGE for the bulk measured
    strictly worse in-context (engines switch queues at coarse packet
    boundaries; a deeply queued SWDGE stream starves everything else,
    and SWDGE itself drops to ~20 GB/s/engine under PE load vs 26 in
    isolation; HWDGE sustains ~24.5 in-context).
  - Conv streams as [128 x 3072] blocks (12KB lines = 3x4KB packets,
    measured ~3% faster per engine than 6KB lines); x/y as 16
    [128 x 1024] chunks, one per conv block over the first 16 blocks.
    Spreading MSE traffic this thin keeps every tensor-engine idle
    stretch under the HAM 3.4us clock-gate window even when the
    shared HBM degrades to ~340 GB/s (with 8KB chunks over 8 blocks,
    a degraded run cooled the PE repeatedly and added 5us of drain).
  - Each conv tile carries 64 junk columns so the last sub-tile's
    256-wide fp32r moving operand stays in bounds; junk only feeds
    PSUM columns 192:255, which are never read.  No memset.
  - The final conv blocks shrink (16,...,8,4,2,2 tiles) so the PE
    drains right after the last byte lands.
  - Per 128-row sub-tile: two fp32r full-rate matmuls accumulate
    C = A^T A into cps1/cps2 across the whole shard.
  - Tail: two DVE copies move C out of PSUM into the merged output
    tile, then a single DMA returns [128, 388]: cols 0:192 = C rows
    0:128, cols 192:384 = C rows 128:192 (partitions 0:64), cols
    384:388 = MSE partial sums.
"""

import numpy as np

import concourse.bass as bass
import concourse.mybir as mybir
from concourse import bacc, tile
from concourse.bass_utils import run_bass_kernel_spmd

N_CORES = 8
B, D = 2048, 4096
M, G = 262144, 192
F, KW = 64, 3
ROWS = B // N_CORES          # 256
MC = M // N_CORES            # 32768
NT = MC // 128               # 256 sub-tiles of [128, 192]
PAD = 64                     # junk cols for the fp32r 256-wide moving op
TPBS = [16] * 14 + [8, 8, 8, 4, 2, 2]  # tiles per conv block
assert sum(TPBS) == NT
NCH = 8                      # MSE chunks per operand
CHW = (ROWS * D) // (128 * NCH)  # 1024 cols per partition per chunk
OC = 2 * G + NCH             # merged output cols: C(0:128), C(128:192), sse

ALPHA = 0.0005
TAU = 0.2

_prog = None


def _build() -> bass.Bass:
    nc = bacc.Bacc(None, target_bir_lowering=False)
    f32 = mybir.dt.float32
    f32r = mybir.dt.float32r

    xs = nc.dram_tensor("xs", [ROWS, D], f32, kind="ExternalInput")
    ys = nc.dram_tensor("ys", [ROWS, D], f32, kind="ExternalInput")
    aw = nc.dram_tensor("aw", [MC, G], f32r, kind="ExternalInput")
    out = nc.dram_tensor("out", [128, OC], f32, kind="ExternalOutput")

    RW = 256  # moving operand width for the fp32r full-rate mode

    with tile.TileContext(nc) as tc:
        with (
            tc.tile_pool(name="apool", bufs=8) as apool,
            tc.tile_pool(name="xpool", bufs=2) as xpool,
            tc.tile_pool(name="ypool", bufs=2) as ypool,
            tc.tile_pool(name="dpool", bufs=2) as dpool,
            tc.tile_pool(name="opool", bufs=1) as opool,
            tc.tile_pool(name="psum", bufs=1, space=bass.MemorySpace.PSUM) as psum,
        ):
            cps1 = psum.tile([128, RW], f32, tag="cps1")
            cps2 = psum.tile([F, RW], f32, tag="cps2")
            otile = opool.tile([128, OC], f32, tag="otile")

            awv = aw[:].rearrange("(p t) g -> p (t g)", p=128)
            xv = xs[:].rearrange("(p t) d -> p (t d)", p=128)
            yv = ys[:].rearrange("(p t) d -> p (t d)", p=128)

            ti = 0
            c0 = 0
            for b, tpb in enumerate(TPBS):
                at = apool.tile([128, 16 * G + PAD], f32r, tag="at")
                nc.sync.dma_start(at[:, :tpb * G], awv[:, c0:c0 + tpb * G])
                c0 += tpb * G

                # one x or y chunk per block over the first 8 blocks
                if b < 2 * NCH:
                    ch = b // 2
                    if b % 2 == 0:
                        xt = xpool.tile([128, CHW], f32, tag="xt")
                        nc.sync.dma_start(xt[:], xv[:, ch * CHW:(ch + 1) * CHW])
                    else:
                        yt = ypool.tile([128, CHW], f32, tag="yt")
                        nc.sync.dma_start(yt[:], yv[:, ch * CHW:(ch + 1) * CHW])

                for t in range(tpb):
                    rhs = at[:, t * G:t * G + RW]
                    w1 = at[:, t * G:t * G + 128]
                    w2 = at[:, t * G + 128:t * G + G]
                    nc.tensor.matmul(
                        cps1[:], w1, rhs,
                        start=(ti == 0), stop=(ti == NT - 1),
                    )
                    nc.tensor.matmul(
                        cps2[:], w2, rhs,
                        start=(ti == 0), stop=(ti == NT - 1),
                    )
                    ti += 1

                if b < 2 * NCH and b % 2 == 1:
                    ch = b // 2
                    dtile = dpool.tile([128, CHW], f32, tag="dt")
                    nc.vector.tensor_sub(dtile[:], xt[:], yt[:])
                    nc.scalar.activation(
                        dtile[:], dtile[:],
                        mybir.ActivationFunctionType.Square,
                        accum_out=otile[:, 2 * G + ch:2 * G + ch + 1],
                    )

            # tail: move C out of PSUM and ship it raw; the host does the
            # k-diagonal gather
            nc.vector.tensor_copy(otile[:, 0:G], cps1[:, :G])
            nc.vector.tensor_copy(otile[0:F, G:2 * G], cps2[:, :G])
            nc.sync.dma_start(out[:], otile[:])

    nc.finalize()
    return nc


def _get_prog() -> bass.Bass:
    global _prog
    if _prog is None:
        _prog = _build()
    return _prog


def _epilogue(C: np.ndarray, sse: float) -> np.ndarray:
    # gram[i,j] = sum_k C[3i+k, 3j+k]
    Cr = C.reshape(F, KW, F, KW)
    gram = sum(Cr[:, k, :, k] for k in range(KW))
    norms = np.sqrt(np.diag(gram))
    sim = gram / np.outer(norms, norms)
    mask = (sim > TAU) & (sim <= 1.0) & (~np.eye(F, dtype=bool))
    reg = sim[mask].sum()
    loss = sse / float(B * D) + ALPHA * reg
    return np.asarray(np.float32(loss))


def kernel(x_batch: np.ndarray, y_batch: np.ndarray, conv_w: np.ndarray) -> np.ndarray:
    nc = _get_prog()
    A = np.ascontiguousarray(conv_w.reshape(M, G))
    in_maps = []
    for c in range(N_CORES):
        in_maps.append({
            "xs": np.ascontiguousarray(x_batch[c * ROWS:(c + 1) * ROWS]),
            "ys": np.ascontiguousarray(y_batch[c * ROWS:(c + 1) * ROWS]),
            "aw": np.ascontiguousarray(A[c * MC:(c + 1) * MC]),
        })
    res = run_bass_kernel_spmd(nc, in_maps, core_ids=list(range(N_CORES))).results
    C = np.zeros((G, G), np.float64)
    sse = 0.0
    for r in res:
        o = r["out"]
        C[0:128] += o[:, 0:G].astype(np.float64)
        C[128:G] += o[0:F, G:2 * G].astype(np.float64)
        sse += float(o[:, 2 * G:2 * G + NCH].sum(dtype=np.float64))
    return _epilogue(C, sse)
